# revision 41
# baseline (speedup 1.0000x reference)
"""Trainium2 Bass kernel for CustomSelfAttentionWithBias (B=2, T=2048, C=1024, H=16).

Computes y = proj(softmax(mask(QK^T/sqrt(hd) + emphasis_col0)) @ V) where
qkv = x @ W_attn, with a causal bool mask and +1.0 emphasis on score column 0.

Sharding: 8 cores; core c handles batch b = c//4 and heads 4*(c%4) .. +4
(data parallel on B, tensor parallel on heads; c_proj row-sharded so each
core emits a partial y[b] that the host sums).

v13 design notes (233us v2 -> ~168us):
  - The steady state is a ridge between the PE stream (~854ns of
    score+PV per 128-key chunk) and the ACT exp (~1.06us per chunk);
    any PE idle gap resets the DVFS clock (2.4GHz -> 1.2GHz for the
    next 3us), so qkv-gen/proj groups are woven as filler between
    attention chunks. Block 3 has no gen left and a ~7us exp-vs-PE
    deficit, so it gets TWO blocks' proj fillers (proj of blocks 1+2).
  - Norm chain: den row copy on DVE, partition_broadcast on the (idle)
    gpsimd/Pool engine instead of a broadcast DMA (a DMA hop costs
    ~2.5-6.5us issue-to-ready; this was ~8us of PE stall per pair
    boundary in v2). Per-head chains, NOT one merged [1,1024] chain: a
    merged chain keeps the po banks occupied ~2us longer and stalls the
    po ring WAR on short early pairs. Muls stay on DVE (custom-DVE
    reciprocal writes are not cross-engine tracked).
  - Tail: the last pair's den broadcast is a 1-contraction PE matmul
    into the older (free) po banks; recips write SBUF (one-PSUM-operand
    rule); the last block's projection contracts the final head-pair in
    two 64-row steps (direct ot half + the osh tile itself) so nothing
    waits on the final partition-moving osh DMA, and its pair-0 steps
    are queued before the norm ops to stream during the chain.
  - Startup: wq whole, then xt[0] in per-cc slices all on the sync
    queue (DMA bandwidth is shared: anything issued before xt0 delays
    it; slices complete independently and gen consumes them in order).
    The gpsimd library ucode load is kept out of that window.
  - PSUM: 2x[128,1024] score tiles + 4x[128,512] po tiles = 8 banks.
    Whole filler groups only: holding a score-ring tile open across
    separately-paced half-units serializes the ring (v4-v8 regression).
  - Engine budget per core: PE ~131us busy (105us of stream cycles at
    2.4GHz + DVFS tax), ACT exp ~74us, DVE ~80us, Pool ~16us.
"""

import math
import numpy as np
import ml_dtypes

B, T, C = 2, 2048, 1024
H, HD = 16, 64
NH = 4            # heads per core
N_CORES = 8
QB = 512          # query block (columns of S^T per matmul)
KC = 128          # key chunk (partition dim of S^T)
N_QB = T // QB    # 4
N_KC = T // KC    # 16
CCH = C // 128    # 8 contraction chunks for the projections
EMPHASIS = 1.0
PEND = 3          # PV pending depth (chunks between QK and PV emission)
BISECT_NOFILL = False  # fillers woven between attention chunks

_COMPILED = {}


def _build(causal: bool = True):
    import concourse.bass as bass
    import concourse.tile as tile
    import concourse.mybir as mybir
    from concourse import bacc
    from concourse import library_config

    f32 = mybir.dt.float32
    f16 = mybir.dt.float16
    bf16 = mybir.dt.bfloat16
    EXP = mybir.ActivationFunctionType.Exp

    nc = bacc.Bacc("TRN2", target_bir_lowering=False, debug=False)

    xtr = nc.dram_tensor("xtr", [128, CCH, T], bf16, kind="ExternalInput").ap()
    wq = nc.dram_tensor("wq", [C, NH * HD], bf16, kind="ExternalInput").ap()
    wk = nc.dram_tensor("wk", [C, NH * HD], bf16, kind="ExternalInput").ap()
    wv = nc.dram_tensor("wv", [C, NH * HD], bf16, kind="ExternalInput").ap()
    wp = nc.dram_tensor("wp", [NH * HD, C], bf16, kind="ExternalInput").ap()
    # tng[p, k] = -50 if k > p else 0; eye = identity. A PE matmul
    # tng.T @ eye accumulates the strict-upper -50 mask bias onto the
    # diagonal score strip (exp then underflows masked entries to 0),
    # replacing the per-chunk DVE triangle multiplies.
    tng = nc.dram_tensor("tng", [128, 128], bf16, kind="ExternalInput").ap()
    eye = nc.dram_tensor("eye", [128, 128], bf16, kind="ExternalInput").ap()
    y = nc.dram_tensor("y", [T, C], f16, kind="ExternalOutput").ap()

    with tile.TileContext(nc) as tc:
        _body(nc, tc, bass, mybir, library_config, xtr, wq, wk, wv, wp,
              tng, eye, y, causal, f32, f16, bf16, EXP)
    nc.compile()
    return nc


def _body(nc, tc, bass, mybir, library_config, xtr, wq, wk, wv, wp, tng,
          eye, y, causal, f32, f16, bf16, EXP):
    from contextlib import ExitStack

    ctx = ExitStack()
    singles = ctx.enter_context(tc.tile_pool(name="singles", bufs=1))
    # scores + wide filler groups (gen/proj): 2 x 2 banks
    ps_st = ctx.enter_context(tc.tile_pool(name="ps_st", bufs=2, space="PSUM"))
    # PV accumulator pairs (2 pairs in flight): 4 x 1 banks
    ps_po = ctx.enter_context(tc.tile_pool(name="ps_po", bufs=4, space="PSUM"))
    pt_pool = ctx.enter_context(tc.tile_pool(name="pt_pool", bufs=PEND + 2))
    rec_pool = ctx.enter_context(tc.tile_pool(name="rec_pool", bufs=4))
    rs_pool = ctx.enter_context(tc.tile_pool(name="rs_pool", bufs=2))
    osh_pool = ctx.enter_context(tc.tile_pool(name="osh_pool", bufs=2))
    y_pool = ctx.enter_context(tc.tile_pool(name="y_pool", bufs=3))

    # ---- resident SBUF tiles --------------------------------------------
    wq_sb = singles.tile([128, CCH, NH * HD], bf16, name="wq_sb")
    wk_sb = singles.tile([128, CCH, NH * HD], bf16, name="wk_sb")
    wv_sb = singles.tile([128, CCH, NH * HD], bf16, name="wv_sb")
    wp_sb = singles.tile([128, 2, C], bf16, name="wp_sb")
    # second head of pair 1's Wp rows staged at partitions 0:64 (tail proj)
    wpb2_sb = singles.tile([HD, C], bf16, name="wpb2_sb")
    tng_sb = singles.tile([128, 128], bf16, name="tng_sb")
    eye_sb = singles.tile([128, 128], bf16, name="eye_sb")
    # ones column for the tail's PE-matmul den broadcast
    ones_sb = singles.tile([1, HD], bf16, name="ones_sb")
    # xT per t-block: [128, cc, 512]
    xt_t = [singles.tile([128, CCH, QB], bf16, name=f"xt{i}")
            for i in range(N_QB)]
    # Q^T / K^T per (head pair, t-block): [128 = 2 heads x 64, 512]
    qt_t = [[singles.tile([128, QB], bf16, name=f"qt{p}_{i}")
             for i in range(N_QB)] for p in range(2)]
    kt_t = [[singles.tile([128, QB], bf16, name=f"kt{p}_{i}")
             for i in range(N_QB)] for p in range(2)]
    # V|ones per kc pair: [128 k, 2, head, 65]
    v_t = [singles.tile([128, 2, NH, HD + 1], bf16, name=f"v{j}")
           for j in range(N_KC // 2)]
    # O^T per (head pair, q-block)
    ot_t = [[singles.tile([128, QB], bf16, name=f"ot{p}_{i}")
             for i in range(N_QB)] for p in range(2)]

    # ---- input DMAs: wq first, then xt0 in per-cc slices (DMA bandwidth
    # is shared, so later transfers delay xt0; slices complete
    # independently and the gen matmuls consume them in cc order) --------
    nc.sync.dma_start(out=wq_sb, in_=wq.rearrange("(c p) n -> p c n", p=128))
    xtr_v = xtr.rearrange("p c (i q) -> p c i q", q=QB)
    for cc in range(CCH):
        nc.sync.dma_start(out=xt_t[0][:, cc, :], in_=xtr_v[:, cc, 0, :])
    nc.sync.dma_start(out=wk_sb, in_=wk.rearrange("(c p) n -> p c n", p=128))
    nc.sync.dma_start(out=wv_sb, in_=wv.rearrange("(c p) n -> p c n", p=128))
    for i in range(1, N_QB):
        nc.sync.dma_start(out=xt_t[i], in_=xtr[:, :, i * QB:(i + 1) * QB])
    nc.sync.dma_start(out=wp_sb, in_=wp.rearrange("(j p) n -> p j n", p=128))
    nc.sync.dma_start(out=tng_sb, in_=tng)
    nc.sync.dma_start(out=eye_sb, in_=eye)
    nc.sync.dma_start(out=wpb2_sb, in_=wp[3 * HD:4 * HD, :])
    # the library ucode load is itself a DRAM read: keep it out of the
    # critical wq/xt0 bandwidth window (first use is the first pair norm)
    nc.gpsimd.load_library(library_config.attn)
    for j in range(N_KC // 2):
        nc.vector.memset(v_t[j][:, :, :, HD:HD + 1], 1.0)
    nc.vector.memset(ones_sb, 1.0)

    # ---- filler units (each ~0.9-1.8us of PE work) ----------------------
    def cast(eng, out, in_):
        if eng is nc.scalar:
            eng.copy(out, in_)
        else:
            eng.tensor_copy(out, in_)

    def gen_qkw(dst, w_sb, nb, eng):
        # wide: Q^T (or K^T) for BOTH head pairs of t-block nb. One unit
        # = one PSUM-ring tile, fully emitted in one go: holding a ring
        # tile open across separately-paced halves serializes the ring.
        pg = ps_st.tile([128, 2 * QB], f32, tag="st", name="pg_qk")
        for pr in range(2):
            for cc in range(CCH):
                nc.tensor.matmul(
                    pg[:, pr * QB:(pr + 1) * QB],
                    w_sb[:, cc, pr * 128:(pr + 1) * 128],
                    xt_t[nb][:, cc, :],
                    start=(cc == 0), stop=(cc == CCH - 1))
        for pr in range(2):
            cast(eng, dst[pr][nb], pg[:, pr * QB:(pr + 1) * QB])

    def gen_vw(nb, eng):
        # wide: V|ones for the 4 key chunks of t-block nb
        pg = ps_st.tile([128, 2 * QB], f32, tag="st", name="pg_v")
        for q in range(4):
            for cc in range(CCH):
                nc.tensor.matmul(
                    pg[:, q * 256:(q + 1) * 256],
                    xt_t[nb][:, cc, q * 128:(q + 1) * 128],
                    wv_sb[:, cc, :],
                    start=(cc == 0), stop=(cc == CCH - 1))
        for q in range(4):
            cast(
                eng, v_t[2 * nb + q // 2][:, q % 2, :, 0:HD],
                pg[:, q * 256:(q + 1) * 256].rearrange(
                    "p (h n) -> p h n", h=NH))
        if nb == 0:
            # emphasis: P column for k==0 gets exp(s+1); fold e into
            # V|ones (k==0 row of all 4 heads' V and ones columns)
            nc.scalar.mul(v_t[0][0:1, 0, :, :], v_t[0][0:1, 0, :, :],
                          float(math.exp(EMPHASIS)))

    def proj_w(qb, tci, eng):
        # wide: both 512-col halves of y rows [128*(4qb+tci) ..]
        tc_i = 4 * qb + tci
        py = ps_st.tile([128, 2 * QB], f32, tag="st", name="py_w")
        for ch in range(2):
            for pr2 in range(2):
                nc.tensor.matmul(
                    py[:, ch * QB:(ch + 1) * QB],
                    ot_t[pr2][qb][:, tci * 128:(tci + 1) * 128],
                    wp_sb[:, pr2, ch * QB:(ch + 1) * QB],
                    start=(pr2 == 0), stop=(pr2 == 1))
        ysb = y_pool.tile([128, C], f16, tag="ysb", name=f"ysb{tc_i}")
        cast(eng, ysb, py)
        nc.sync.dma_start(out=y[tc_i * 128:(tc_i + 1) * 128, :], in_=ysb)

    def proj_units(qb):
        return [(lambda eng, t=tci: proj_w(qb, t, eng)) for tci in range(4)]

    def gen_units(nb):
        return [lambda eng: gen_qkw(qt_t, wq_sb, nb, eng),
                lambda eng: gen_qkw(kt_t, wk_sb, nb, eng),
                lambda eng: gen_vw(nb, eng)]

    # ---- attention ------------------------------------------------------
    last_osh = {}

    def norm_pair(pr, qb, po0, po1, last):
        if last:
            # tail fast path: bf16 den copies run DVE + ACT in parallel,
            # the broadcast is a 1-contraction PE matmul into the now-free
            # older po banks (0.2us each vs 1.8us on Pool), and recip+mul
            # go per-head so the tail proj's pair-1 steps unblock asap
            rsb = rs_pool.tile([1, 2 * QB], bf16, tag="rsb")
            nc.vector.tensor_copy(rsb[:, 0:QB], po0[HD:HD + 1, :])
            nc.scalar.copy(rsb[:, QB:2 * QB], po1[HD:HD + 1, :])
            recpa = ps_po.tile([128, QB], f32, tag="po", name="recpa")
            recpb = ps_po.tile([128, QB], f32, tag="po", name="recpb")
            nc.tensor.matmul(recpa[0:HD, :], ones_sb, rsb[:, 0:QB],
                             start=True, stop=True)
            nc.tensor.matmul(recpb[0:HD, :], ones_sb, rsb[:, QB:2 * QB],
                             start=True, stop=True)
            # recip PSUM->SBUF: the muls may read only ONE PSUM operand
            reca = rec_pool.tile([HD, QB], f32, tag="reca")
            recb = rec_pool.tile([HD, QB], f32, tag="reca")
            nc.vector.reciprocal_approx_fast(out=reca, in_=recpa[0:HD, :])
            nc.vector.tensor_mul(ot_t[pr][qb][0:HD, :], po0[0:HD, :], reca)
            nc.vector.reciprocal_approx_fast(out=recb, in_=recpb[0:HD, :])
            osh = osh_pool.tile([HD, QB], bf16, tag="osh")
            nc.vector.tensor_mul(osh, po1[0:HD, :], recb)
            last_osh[(pr, qb)] = osh
            return
        # per-head chains (short chains free the po banks asap: a merged
        # [1,1024] chain keeps them occupied ~2us longer and stalls the
        # po ring's WAR on short early pairs): den row PSUM->SBUF on DVE,
        # partition 0 broadcast on gpsimd/Pool (no DMA machinery), then
        # 1/den in place on DVE (the custom-DVE write is only read by the
        # same-engine mul: custom-DVE writes are not cross-engine
        # tracked), then multiply. Head b first so its partition-moving
        # osh DMA launches earlier.
        for s, po in ((1, po1), (0, po0)):
            rs = rs_pool.tile([1, QB], f32, tag="rs")
            nc.vector.tensor_copy(rs, po[HD:HD + 1, :])
            rec = rec_pool.tile([HD, QB], f32, tag="rec")
            nc.gpsimd.partition_broadcast(rec, rs)
            nc.vector.reciprocal_approx_fast(out=rec, in_=rec)
            if s == 0:
                nc.vector.tensor_mul(ot_t[pr][qb][0:HD, :], po[0:HD, :], rec)
            else:
                osh = osh_pool.tile([HD, QB], bf16, tag="osh")
                nc.vector.tensor_mul(osh, po[0:HD, :], rec)
                last_osh[(pr, qb)] = osh
                nc.sync.dma_start(out=ot_t[pr][qb][HD:128, :], in_=osh)

    def emit_pv(rec_, do_norm=True):
        pr, qb, kc, w0, pt, po0, po1, nk = rec_
        v = v_t[kc // 2][:, kc % 2, :, :]
        nc.tensor.matmul(po0[0:HD + 1, w0:QB], v[:, 2 * pr, :],
                         pt[:, w0:QB],
                         start=(kc == 0), stop=(kc == nk - 1))
        nc.tensor.matmul(po1[0:HD + 1, w0:QB], v[:, 2 * pr + 1, :],
                         pt[:, QB + w0:2 * QB],
                         start=(kc == 0), stop=(kc == nk - 1))
        if kc == nk - 1 and do_norm:
            last = (pr == 1 and qb == N_QB - 1)
            norm_pair(pr, qb, po0, po1, last)

    def proj_tail_pre(tci):
        # pair-0 contraction steps depend only on ot_t[0] (normed one
        # pair earlier): emit before the last norm chain to cover it
        qb = N_QB - 1
        py = ps_st.tile([128, 2 * QB], f32, tag="st", name=f"pyt{tci}")
        for ch in range(2):
            sl = slice(ch * QB, (ch + 1) * QB)
            nc.tensor.matmul(py[:, sl],
                             ot_t[0][qb][:, tci * 128:(tci + 1) * 128],
                             wp_sb[:, 0, sl], start=True, stop=False)
        return py

    def proj_tail(tci, eng, py=None):
        # last q-block: contract pair 1 in two 64-row steps (the direct
        # ot half + the osh tile itself at partitions 0:64) so nothing
        # waits on the final partition-moving osh DMA
        qb = N_QB - 1
        tc_i = 4 * qb + tci
        osh = last_osh[(1, qb)]
        if py is None:
            py = proj_tail_pre(tci)
        for ch in range(2):
            sl = slice(ch * QB, (ch + 1) * QB)
            nc.tensor.matmul(py[:, sl],
                             ot_t[1][qb][0:HD, tci * 128:(tci + 1) * 128],
                             wp_sb[0:HD, 1, sl], start=False, stop=False)
            nc.tensor.matmul(py[:, sl],
                             osh[:, tci * 128:(tci + 1) * 128],
                             wpb2_sb[:, sl], start=False, stop=True)
        ysb = y_pool.tile([128, C], f16, tag="ysb", name=f"ysb{tc_i}")
        cast(eng, ysb, py)
        nc.sync.dma_start(out=y[tc_i * 128:(tc_i + 1) * 128, :], in_=ysb)

    # ---- main loop ------------------------------------------------------
    # filler schedule per block: chunk indices at which to emit the next
    # unit. gen halves sharing a PSUM tile are paced adjacently; proj of
    # block qb-1 needs both pairs' norms (~chunk 4-5), so proj due >= 6.
    # block 3 has no gen left and a 6.8us exp-vs-PE deficit, so proj(1)
    # is deferred from block 2 into block 3 alongside proj(2).
    filler_plan = {
        0: (gen_units(1) if N_QB > 1 else [],
            [0, 1, 5]),
        1: (gen_units(2) + proj_units(0),
            [0, 2, 4, 6, 9, 12, 15]),
        2: (gen_units(3),
            [0, 3, 6]),
        3: (proj_units(1) + proj_units(2),
            [0, 4, 8, 12, 6, 14, 22, 30]),
    }
    pending = []

    # gen(0) runs as the prefix; casts on the then-idle DVE
    for u in gen_units(0):
        u(nc.vector)

    for qb in range(N_QB):
        nk = 4 * (qb + 1) if causal else N_KC
        units, due_raw = filler_plan[qb]
        # pair each unit with its due chunk, then order by due (stable:
        # gen halves sharing a PSUM tile keep their relative order)
        pairs = sorted(zip(due_raw, range(len(units))), key=lambda p: p[0])
        due = [p[0] for p in pairs]
        units = [units[p[1]] for p in pairs]
        # filler casts stay on DVE: anything queued on ACT ahead of an
        # exp delays the exp cadence, which is the pipeline clock
        filler_eng = nc.vector
        f_i = 0
        i = 0
        for pr in range(2):
            po0 = ps_po.tile([128, QB], f32, tag="po", name="po0")
            po1 = ps_po.tile([128, QB], f32, tag="po", name="po1")
            for kc in range(nk):
                r = kc - 4 * qb
                w0 = 128 * r if (causal and r > 0) else 0
                st = ps_st.tile([128, 2 * QB], f32, tag="st")
                diag = causal and r >= 0
                for s in range(2):
                    r0, r1 = s * HD, (s + 1) * HD
                    base = s * QB + w0
                    nc.tensor.matmul(
                        st[:, base:(s + 1) * QB],
                        kt_t[pr][kc // 4][r0:r1, (kc % 4) * 128:(kc % 4 + 1) * 128],
                        qt_t[pr][qb][r0:r1, w0:QB],
                        start=True, stop=not diag)
                    if diag:
                        # accumulate -50 onto the masked strict-upper strip
                        nc.tensor.matmul(
                            st[:, base:base + 128], tng_sb, eye_sb,
                            start=False, stop=True)
                pt = pt_pool.tile([128, 2 * QB], bf16, tag="pt")
                if w0 == 0:
                    nc.scalar.activation(out=pt, in_=st, func=EXP)
                else:
                    stv = st.rearrange("p (a q) -> p a q", a=2)
                    ptv = pt.rearrange("p (a q) -> p a q", a=2)
                    nc.scalar.activation(out=ptv[:, :, w0:QB],
                                         in_=stv[:, :, w0:QB], func=EXP)
                pending.append((pr, qb, kc, w0, pt, po0, po1, nk))
                while len(pending) > PEND:
                    emit_pv(pending.pop(0))
                if not BISECT_NOFILL:
                    while f_i < len(units) and i >= due[f_i]:
                        units[f_i](filler_eng)
                        f_i += 1
                i += 1
        while f_i < len(units):
            units[f_i](filler_eng)
            f_i += 1
    # final drain: emit the last PV without its norm, queue the first two
    # tail-proj pair-0 steps on the PE, THEN the norm ops — the pair-0
    # steps stream while the norm chain runs on DVE/ACT/Pool
    while len(pending) > 1:
        emit_pv(pending.pop(0))
    rec_l = pending.pop(0)
    emit_pv(rec_l, do_norm=False)
    pys = {tci: proj_tail_pre(tci) for tci in (0, 1)}
    norm_pair(rec_l[0], rec_l[1], rec_l[5], rec_l[6], True)
    # trailing projection of the last q-block; casts alternate engines so
    # consecutive tiles pipeline
    for tci in range(4):
        proj_tail(tci, nc.scalar if tci % 2 == 0 else nc.vector,
                  pys.get(tci))

    ctx.close()


def _prep_inputs(x, W_attn, W_proj, attn_mask):
    """Host-side shard + layout prep. Returns (in_maps, causal)."""
    bf = ml_dtypes.bfloat16
    causal = bool(np.array_equal(
        np.asarray(attn_mask),
        np.tril(np.ones((T, T), dtype=bool))))

    x = np.asarray(x, dtype=np.float32)
    Wa = np.asarray(W_attn, dtype=np.float32)
    Wp = np.asarray(W_proj, dtype=np.float32)

    scale = 1.0 / np.sqrt(np.float32(HD))
    # [128, cc, T]: partition p holds rows c = cc*128 + p of x[b].T
    xtr_b = [np.ascontiguousarray(
        x[b].T.reshape(CCH, 128, T).transpose(1, 0, 2)).astype(bf)
        for b in range(B)]

    # tng[p, k] = -50 where k > p (strict upper); eye = identity. The
    # kernel accumulates tng.T @ eye onto diagonal score strips so the
    # exp underflows masked entries to 0.
    i = np.arange(128)
    tng = np.where(i[None, :] > i[:, None], -50.0, 0.0).astype(bf)
    eye = np.eye(128, dtype=np.float32).astype(bf)

    in_maps = []
    for core in range(N_CORES):
        b, h0 = core // 4, (core % 4) * NH
        hsl = slice(h0 * HD, (h0 + NH) * HD)
        wq_c = np.ascontiguousarray(Wa[:, hsl] * scale).astype(bf)
        wk_c = np.ascontiguousarray(Wa[:, C + h0 * HD: C + (h0 + NH) * HD]).astype(bf)
        wv_c = np.ascontiguousarray(Wa[:, 2 * C + h0 * HD: 2 * C + (h0 + NH) * HD]).astype(bf)
        wp_c = np.ascontiguousarray(Wp[hsl, :]).astype(bf)
        in_maps.append({
            "xtr": xtr_b[b], "wq": wq_c, "wk": wk_c, "wv": wv_c,
            "wp": wp_c, "tng": tng, "eye": eye,
        })
    return in_maps, causal


def kernel(x, W_attn, W_proj, attn_mask, _trace=False):
    from concourse import bass_utils

    in_maps, causal = _prep_inputs(x, W_attn, W_proj, attn_mask)
    key = ("causal" if causal else "dense")
    if key not in _COMPILED:
        _COMPILED[key] = _build(causal)
    nc = _COMPILED[key]

    res = bass_utils.run_bass_kernel_spmd(
        nc, in_maps, core_ids=list(range(N_CORES)), trace=_trace)

    y = np.zeros((B, T, C), dtype=np.float32)
    for core in range(N_CORES):
        y[core // 4] += res.results[core]["y"].astype(np.float32)
    if _trace:
        kernel._last_results = res
    return y


# revision 42
# speedup vs baseline: 1.0800x; 1.0800x over previous
"""Trainium2 Bass kernel for CustomSelfAttentionWithBias (B=2, T=2048, C=1024, H=16).

Computes y = proj(softmax(mask(QK^T/sqrt(hd) + emphasis_col0)) @ V) where
qkv = x @ W_attn, with a causal bool mask and +1.0 emphasis on score column 0.

Sharding: 8 cores; core c handles batch b = c//4 and heads 4*(c%4) .. +4
(data parallel on B, tensor parallel on heads; c_proj row-sharded so each
core emits a partial y[b] that the host sums).

v13 design notes (233us v2 -> ~168us):
  - The steady state is a ridge between the PE stream (~854ns of
    score+PV per 128-key chunk) and the ACT exp (~1.06us per chunk);
    any PE idle gap resets the DVFS clock (2.4GHz -> 1.2GHz for the
    next 3us), so qkv-gen/proj groups are woven as filler between
    attention chunks. Block 3 has no gen left and a ~7us exp-vs-PE
    deficit, so it gets TWO blocks' proj fillers (proj of blocks 1+2).
  - Norm chain: den row copy on DVE, partition_broadcast on the (idle)
    gpsimd/Pool engine instead of a broadcast DMA (a DMA hop costs
    ~2.5-6.5us issue-to-ready; this was ~8us of PE stall per pair
    boundary in v2). Per-head chains, NOT one merged [1,1024] chain: a
    merged chain keeps the po banks occupied ~2us longer and stalls the
    po ring WAR on short early pairs. Muls stay on DVE (custom-DVE
    reciprocal writes are not cross-engine tracked).
  - Tail: the last pair's den broadcast is a 1-contraction PE matmul
    into the older (free) po banks; recips write SBUF (one-PSUM-operand
    rule); the last block's projection contracts the final head-pair in
    two 64-row steps (direct ot half + the osh tile itself) so nothing
    waits on the final partition-moving osh DMA, and its pair-0 steps
    are queued before the norm ops to stream during the chain.
  - Startup: wq whole, then xt[0] in per-cc slices all on the sync
    queue (DMA bandwidth is shared: anything issued before xt0 delays
    it; slices complete independently and gen consumes them in order).
    The gpsimd library ucode load is kept out of that window.
  - PSUM: 2x[128,1024] score tiles + 4x[128,512] po tiles = 8 banks.
    Whole filler groups only: holding a score-ring tile open across
    separately-paced half-units serializes the ring (v4-v8 regression).
  - Engine budget per core: PE ~131us busy (105us of stream cycles at
    2.4GHz + DVFS tax), ACT exp ~74us, DVE ~80us, Pool ~16us.
"""

import math
import numpy as np
import ml_dtypes

B, T, C = 2, 2048, 1024
H, HD = 16, 64
NH = 4            # heads per core
N_CORES = 8
QB = 512          # query block (columns of S^T per matmul)
KC = 128          # key chunk (partition dim of S^T)
N_QB = T // QB    # 4
N_KC = T // KC    # 16
CCH = C // 128    # 8 contraction chunks for the projections
EMPHASIS = 1.0
PEND = 3          # PV pending depth (chunks between QK and PV emission)
BISECT_NOFILL = False  # fillers woven between attention chunks

_COMPILED = {}


def _build(causal: bool = True):
    import concourse.bass as bass
    import concourse.tile as tile
    import concourse.mybir as mybir
    from concourse import bacc
    from concourse import library_config

    f32 = mybir.dt.float32
    f16 = mybir.dt.float16
    bf16 = mybir.dt.bfloat16
    EXP = mybir.ActivationFunctionType.Exp

    nc = bacc.Bacc("TRN2", target_bir_lowering=False, debug=False)

    xtr = nc.dram_tensor("xtr", [128, CCH, T], bf16, kind="ExternalInput").ap()
    wq = nc.dram_tensor("wq", [C, NH * HD], bf16, kind="ExternalInput").ap()
    wk = nc.dram_tensor("wk", [C, NH * HD], bf16, kind="ExternalInput").ap()
    wv = nc.dram_tensor("wv", [C, NH * HD], bf16, kind="ExternalInput").ap()
    wp = nc.dram_tensor("wp", [NH * HD, C], bf16, kind="ExternalInput").ap()
    tri = nc.dram_tensor("tri", [128, 128], bf16, kind="ExternalInput").ap()
    y = nc.dram_tensor("y", [T, C], f16, kind="ExternalOutput").ap()

    with tile.TileContext(nc) as tc:
        _body(nc, tc, bass, mybir, library_config, xtr, wq, wk, wv, wp, tri,
              y, causal, f32, f16, bf16, EXP)
    nc.compile()
    return nc


def _body(nc, tc, bass, mybir, library_config, xtr, wq, wk, wv, wp, tri, y,
          causal, f32, f16, bf16, EXP):
    from contextlib import ExitStack

    ctx = ExitStack()
    singles = ctx.enter_context(tc.tile_pool(name="singles", bufs=1))
    # scores + wide filler groups (gen/proj): 2 x 2 banks
    ps_st = ctx.enter_context(tc.tile_pool(name="ps_st", bufs=2, space="PSUM"))
    # PV accumulator pairs (2 pairs in flight): 4 x 1 banks
    ps_po = ctx.enter_context(tc.tile_pool(name="ps_po", bufs=4, space="PSUM"))
    pt_pool = ctx.enter_context(tc.tile_pool(name="pt_pool", bufs=PEND + 2))
    rec_pool = ctx.enter_context(tc.tile_pool(name="rec_pool", bufs=4))
    rs_pool = ctx.enter_context(tc.tile_pool(name="rs_pool", bufs=2))
    osh_pool = ctx.enter_context(tc.tile_pool(name="osh_pool", bufs=2))
    y_pool = ctx.enter_context(tc.tile_pool(name="y_pool", bufs=3))

    # ---- resident SBUF tiles --------------------------------------------
    wq_sb = singles.tile([128, CCH, NH * HD], bf16, name="wq_sb")
    wk_sb = singles.tile([128, CCH, NH * HD], bf16, name="wk_sb")
    wv_sb = singles.tile([128, CCH, NH * HD], bf16, name="wv_sb")
    wp_sb = singles.tile([128, 2, C], bf16, name="wp_sb")
    # second head of pair 1's Wp rows staged at partitions 0:64 (tail proj)
    wpb2_sb = singles.tile([HD, C], bf16, name="wpb2_sb")
    tri_sb = singles.tile([128, 128], bf16, name="tri_sb")
    # ones column for the tail's PE-matmul den broadcast
    ones_sb = singles.tile([1, HD], bf16, name="ones_sb")
    # xT per t-block: [128, cc, 512]
    xt_t = [singles.tile([128, CCH, QB], bf16, name=f"xt{i}")
            for i in range(N_QB)]
    # Q^T / K^T per (head pair, t-block): [128 = 2 heads x 64, 512]
    qt_t = [[singles.tile([128, QB], bf16, name=f"qt{p}_{i}")
             for i in range(N_QB)] for p in range(2)]
    kt_t = [[singles.tile([128, QB], bf16, name=f"kt{p}_{i}")
             for i in range(N_QB)] for p in range(2)]
    # V|ones per kc pair: [128 k, 2, head, 65]
    v_t = [singles.tile([128, 2, NH, HD + 1], bf16, name=f"v{j}")
           for j in range(N_KC // 2)]
    # O^T per (head pair, q-block)
    ot_t = [[singles.tile([128, QB], bf16, name=f"ot{p}_{i}")
             for i in range(N_QB)] for p in range(2)]

    # ---- input DMAs: wq first, then xt0 in per-cc slices (DMA bandwidth
    # is shared, so later transfers delay xt0; slices complete
    # independently and the gen matmuls consume them in cc order) --------
    nc.sync.dma_start(out=wq_sb, in_=wq.rearrange("(c p) n -> p c n", p=128))
    xtr_v = xtr.rearrange("p c (i q) -> p c i q", q=QB)
    for cc in range(CCH):
        nc.sync.dma_start(out=xt_t[0][:, cc, :], in_=xtr_v[:, cc, 0, :])
    nc.sync.dma_start(out=wk_sb, in_=wk.rearrange("(c p) n -> p c n", p=128))
    nc.sync.dma_start(out=wv_sb, in_=wv.rearrange("(c p) n -> p c n", p=128))
    for i in range(1, N_QB):
        nc.sync.dma_start(out=xt_t[i], in_=xtr[:, :, i * QB:(i + 1) * QB])
    nc.sync.dma_start(out=wp_sb, in_=wp.rearrange("(j p) n -> p j n", p=128))
    nc.sync.dma_start(out=tri_sb, in_=tri)
    nc.sync.dma_start(out=wpb2_sb, in_=wp[3 * HD:4 * HD, :])
    # the library ucode load is itself a DRAM read: keep it out of the
    # critical wq/xt0 bandwidth window (first use is the first pair norm)
    nc.gpsimd.load_library(library_config.attn)
    for j in range(N_KC // 2):
        nc.vector.memset(v_t[j][:, :, :, HD:HD + 1], 1.0)
    nc.vector.memset(ones_sb, 1.0)

    # ---- filler units (each ~0.9-1.8us of PE work) ----------------------
    def cast(eng, out, in_):
        if eng is nc.scalar:
            eng.copy(out, in_)
        else:
            eng.tensor_copy(out, in_)

    def gen_qkw(dst, w_sb, nb, eng):
        # wide: Q^T (or K^T) for BOTH head pairs of t-block nb. One unit
        # = one PSUM-ring tile, fully emitted in one go: holding a ring
        # tile open across separately-paced halves serializes the ring.
        pg = ps_st.tile([128, 2 * QB], f32, tag="st", name="pg_qk")
        for pr in range(2):
            for cc in range(CCH):
                nc.tensor.matmul(
                    pg[:, pr * QB:(pr + 1) * QB],
                    w_sb[:, cc, pr * 128:(pr + 1) * 128],
                    xt_t[nb][:, cc, :],
                    start=(cc == 0), stop=(cc == CCH - 1))
        for pr in range(2):
            cast(eng, dst[pr][nb], pg[:, pr * QB:(pr + 1) * QB])

    def gen_vw(nb, eng):
        # wide: V|ones for the 4 key chunks of t-block nb
        pg = ps_st.tile([128, 2 * QB], f32, tag="st", name="pg_v")
        for q in range(4):
            for cc in range(CCH):
                nc.tensor.matmul(
                    pg[:, q * 256:(q + 1) * 256],
                    xt_t[nb][:, cc, q * 128:(q + 1) * 128],
                    wv_sb[:, cc, :],
                    start=(cc == 0), stop=(cc == CCH - 1))
        for q in range(4):
            cast(
                eng, v_t[2 * nb + q // 2][:, q % 2, :, 0:HD],
                pg[:, q * 256:(q + 1) * 256].rearrange(
                    "p (h n) -> p h n", h=NH))
        if nb == 0:
            # emphasis: P column for k==0 gets exp(s+1); fold e into
            # V|ones (k==0 row of all 4 heads' V and ones columns)
            nc.scalar.mul(v_t[0][0:1, 0, :, :], v_t[0][0:1, 0, :, :],
                          float(math.exp(EMPHASIS)))

    def proj_w(qb, tci, eng):
        # wide: both 512-col halves of y rows [128*(4qb+tci) ..]
        tc_i = 4 * qb + tci
        py = ps_st.tile([128, 2 * QB], f32, tag="st", name="py_w")
        for ch in range(2):
            for pr2 in range(2):
                nc.tensor.matmul(
                    py[:, ch * QB:(ch + 1) * QB],
                    ot_t[pr2][qb][:, tci * 128:(tci + 1) * 128],
                    wp_sb[:, pr2, ch * QB:(ch + 1) * QB],
                    start=(pr2 == 0), stop=(pr2 == 1))
        ysb = y_pool.tile([128, C], f16, tag="ysb", name=f"ysb{tc_i}")
        cast(eng, ysb, py)
        nc.sync.dma_start(out=y[tc_i * 128:(tc_i + 1) * 128, :], in_=ysb)

    def proj_units(qb):
        return [(lambda eng, t=tci: proj_w(qb, t, eng)) for tci in range(4)]

    def gen_units(nb):
        return [lambda eng: gen_qkw(qt_t, wq_sb, nb, eng),
                lambda eng: gen_qkw(kt_t, wk_sb, nb, eng),
                lambda eng: gen_vw(nb, eng)]

    # ---- attention ------------------------------------------------------
    last_osh = {}

    def norm_pair(pr, qb, po0, po1, last):
        if last:
            # tail fast path: bf16 den copies run DVE + ACT in parallel,
            # the broadcast is a 1-contraction PE matmul into the now-free
            # older po banks (0.2us each vs 1.8us on Pool), and recip+mul
            # go per-head so the tail proj's pair-1 steps unblock asap
            rsb = rs_pool.tile([1, 2 * QB], bf16, tag="rsb")
            nc.vector.tensor_copy(rsb[:, 0:QB], po0[HD:HD + 1, :])
            nc.scalar.copy(rsb[:, QB:2 * QB], po1[HD:HD + 1, :])
            recpa = ps_po.tile([128, QB], f32, tag="po", name="recpa")
            recpb = ps_po.tile([128, QB], f32, tag="po", name="recpb")
            nc.tensor.matmul(recpa[0:HD, :], ones_sb, rsb[:, 0:QB],
                             start=True, stop=True)
            nc.tensor.matmul(recpb[0:HD, :], ones_sb, rsb[:, QB:2 * QB],
                             start=True, stop=True)
            # recip PSUM->SBUF: the muls may read only ONE PSUM operand
            reca = rec_pool.tile([HD, QB], f32, tag="reca")
            recb = rec_pool.tile([HD, QB], f32, tag="reca")
            nc.vector.reciprocal_approx_fast(out=reca, in_=recpa[0:HD, :])
            nc.vector.tensor_mul(ot_t[pr][qb][0:HD, :], po0[0:HD, :], reca)
            nc.vector.reciprocal_approx_fast(out=recb, in_=recpb[0:HD, :])
            osh = osh_pool.tile([HD, QB], bf16, tag="osh")
            nc.vector.tensor_mul(osh, po1[0:HD, :], recb)
            last_osh[(pr, qb)] = osh
            return
        # per-head chains (short chains free the po banks asap: a merged
        # [1,1024] chain keeps them occupied ~2us longer and stalls the
        # po ring's WAR on short early pairs): den row PSUM->SBUF on DVE,
        # partition 0 broadcast on gpsimd/Pool (no DMA machinery), then
        # 1/den in place on DVE (the custom-DVE write is only read by the
        # same-engine mul: custom-DVE writes are not cross-engine
        # tracked), then multiply. Head b first so its partition-moving
        # osh DMA launches earlier.
        for s, po in ((1, po1), (0, po0)):
            rs = rs_pool.tile([1, QB], f32, tag="rs")
            nc.vector.tensor_copy(rs, po[HD:HD + 1, :])
            rec = rec_pool.tile([HD, QB], f32, tag="rec")
            nc.gpsimd.partition_broadcast(rec, rs)
            nc.vector.reciprocal_approx_fast(out=rec, in_=rec)
            if s == 0:
                nc.vector.tensor_mul(ot_t[pr][qb][0:HD, :], po[0:HD, :], rec)
            else:
                osh = osh_pool.tile([HD, QB], bf16, tag="osh")
                nc.vector.tensor_mul(osh, po[0:HD, :], rec)
                last_osh[(pr, qb)] = osh
                nc.sync.dma_start(out=ot_t[pr][qb][HD:128, :], in_=osh)

    def emit_pv(rec_, do_norm=True):
        pr, qb, kc, w0, pt, po0, po1, nk = rec_
        v = v_t[kc // 2][:, kc % 2, :, :]
        nc.tensor.matmul(po0[0:HD + 1, w0:QB], v[:, 2 * pr, :],
                         pt[:, w0:QB],
                         start=(kc == 0), stop=(kc == nk - 1))
        nc.tensor.matmul(po1[0:HD + 1, w0:QB], v[:, 2 * pr + 1, :],
                         pt[:, QB + w0:2 * QB],
                         start=(kc == 0), stop=(kc == nk - 1))
        if kc == nk - 1 and do_norm:
            last = (pr == 1 and qb == N_QB - 1)
            norm_pair(pr, qb, po0, po1, last)

    def proj_tail_pre(tci):
        # pair-0 contraction steps depend only on ot_t[0] (normed one
        # pair earlier): emit before the last norm chain to cover it
        qb = N_QB - 1
        py = ps_st.tile([128, 2 * QB], f32, tag="st", name=f"pyt{tci}")
        for ch in range(2):
            sl = slice(ch * QB, (ch + 1) * QB)
            nc.tensor.matmul(py[:, sl],
                             ot_t[0][qb][:, tci * 128:(tci + 1) * 128],
                             wp_sb[:, 0, sl], start=True, stop=False)
        return py

    def proj_tail(tci, eng, py=None):
        # last q-block: contract pair 1 in two 64-row steps (the direct
        # ot half + the osh tile itself at partitions 0:64) so nothing
        # waits on the final partition-moving osh DMA
        qb = N_QB - 1
        tc_i = 4 * qb + tci
        osh = last_osh[(1, qb)]
        if py is None:
            py = proj_tail_pre(tci)
        for ch in range(2):
            sl = slice(ch * QB, (ch + 1) * QB)
            nc.tensor.matmul(py[:, sl],
                             ot_t[1][qb][0:HD, tci * 128:(tci + 1) * 128],
                             wp_sb[0:HD, 1, sl], start=False, stop=False)
            nc.tensor.matmul(py[:, sl],
                             osh[:, tci * 128:(tci + 1) * 128],
                             wpb2_sb[:, sl], start=False, stop=True)
        ysb = y_pool.tile([128, C], f16, tag="ysb", name=f"ysb{tc_i}")
        cast(eng, ysb, py)
        nc.sync.dma_start(out=y[tc_i * 128:(tc_i + 1) * 128, :], in_=ysb)

    # ---- main loop ------------------------------------------------------
    # filler schedule per block: chunk indices at which to emit the next
    # unit. gen halves sharing a PSUM tile are paced adjacently; proj of
    # block qb-1 needs both pairs' norms (~chunk 4-5), so proj due >= 6.
    # block 3 has no gen left and a 6.8us exp-vs-PE deficit, so proj(1)
    # is deferred from block 2 into block 3 alongside proj(2).
    filler_plan = {
        0: (gen_units(1) if N_QB > 1 else [],
            [0, 1, 5]),
        1: (gen_units(2) + proj_units(0),
            [0, 2, 4, 6, 9, 12, 15]),
        2: (gen_units(3),
            [0, 3, 6]),
        3: (proj_units(1) + proj_units(2),
            [0, 4, 8, 12, 6, 14, 22, 30]),
    }
    pending = []

    # gen(0) runs as the prefix; casts on the then-idle DVE
    for u in gen_units(0):
        u(nc.vector)

    for qb in range(N_QB):
        nk = 4 * (qb + 1) if causal else N_KC
        units, due_raw = filler_plan[qb]
        # pair each unit with its due chunk, then order by due (stable:
        # gen halves sharing a PSUM tile keep their relative order)
        pairs = sorted(zip(due_raw, range(len(units))), key=lambda p: p[0])
        due = [p[0] for p in pairs]
        units = [units[p[1]] for p in pairs]
        # filler casts stay on DVE: anything queued on ACT ahead of an
        # exp delays the exp cadence, which is the pipeline clock
        filler_eng = nc.vector
        f_i = 0
        i = 0
        for pr in range(2):
            po0 = ps_po.tile([128, QB], f32, tag="po", name="po0")
            po1 = ps_po.tile([128, QB], f32, tag="po", name="po1")
            for kc in range(nk):
                r = kc - 4 * qb
                w0 = 128 * r if (causal and r > 0) else 0
                st = ps_st.tile([128, 2 * QB], f32, tag="st")
                for s in range(2):
                    r0, r1 = s * HD, (s + 1) * HD
                    nc.tensor.matmul(
                        st[:, s * QB + w0:(s + 1) * QB],
                        kt_t[pr][kc // 4][r0:r1, (kc % 4) * 128:(kc % 4 + 1) * 128],
                        qt_t[pr][qb][r0:r1, w0:QB],
                        start=True, stop=True)
                pt = pt_pool.tile([128, 2 * QB], bf16, tag="pt")
                if w0 == 0:
                    nc.scalar.activation(out=pt, in_=st, func=EXP)
                else:
                    stv = st.rearrange("p (a q) -> p a q", a=2)
                    ptv = pt.rearrange("p (a q) -> p a q", a=2)
                    nc.scalar.activation(out=ptv[:, :, w0:QB],
                                         in_=stv[:, :, w0:QB], func=EXP)
                if causal and r >= 0:
                    for s in range(2):
                        nc.vector.tensor_mul(
                            pt[:, s * QB + w0:s * QB + w0 + 128],
                            pt[:, s * QB + w0:s * QB + w0 + 128],
                            tri_sb)
                pending.append((pr, qb, kc, w0, pt, po0, po1, nk))
                while len(pending) > PEND:
                    emit_pv(pending.pop(0))
                if not BISECT_NOFILL:
                    while f_i < len(units) and i >= due[f_i]:
                        units[f_i](filler_eng)
                        f_i += 1
                i += 1
        while f_i < len(units):
            units[f_i](filler_eng)
            f_i += 1
    # final drain: emit the last PV without its norm, queue the first two
    # tail-proj pair-0 steps on the PE, THEN the norm ops — the pair-0
    # steps stream while the norm chain runs on DVE/ACT/Pool
    while len(pending) > 1:
        emit_pv(pending.pop(0))
    rec_l = pending.pop(0)
    emit_pv(rec_l, do_norm=False)
    pys = {tci: proj_tail_pre(tci) for tci in (0, 1)}
    norm_pair(rec_l[0], rec_l[1], rec_l[5], rec_l[6], True)
    # trailing projection of the last q-block; casts alternate engines so
    # consecutive tiles pipeline
    for tci in range(4):
        proj_tail(tci, nc.scalar if tci % 2 == 0 else nc.vector,
                  pys.get(tci))

    ctx.close()


def _prep_inputs(x, W_attn, W_proj, attn_mask):
    """Host-side shard + layout prep. Returns (in_maps, causal)."""
    bf = ml_dtypes.bfloat16
    causal = bool(np.array_equal(
        np.asarray(attn_mask),
        np.tril(np.ones((T, T), dtype=bool))))

    x = np.asarray(x, dtype=np.float32)
    Wa = np.asarray(W_attn, dtype=np.float32)
    Wp = np.asarray(W_proj, dtype=np.float32)

    scale = 1.0 / np.sqrt(np.float32(HD))
    # [128, cc, T]: partition p holds rows c = cc*128 + p of x[b].T
    xtr_b = [np.ascontiguousarray(
        x[b].T.reshape(CCH, 128, T).transpose(1, 0, 2)).astype(bf)
        for b in range(B)]

    # in-stripe causal triangle: tri[k, q] = 1.0 if k <= q else 0
    i = np.arange(128)
    tri = (i[:, None] <= i[None, :]).astype(bf)

    in_maps = []
    for core in range(N_CORES):
        b, h0 = core // 4, (core % 4) * NH
        hsl = slice(h0 * HD, (h0 + NH) * HD)
        wq_c = np.ascontiguousarray(Wa[:, hsl] * scale).astype(bf)
        wk_c = np.ascontiguousarray(Wa[:, C + h0 * HD: C + (h0 + NH) * HD]).astype(bf)
        wv_c = np.ascontiguousarray(Wa[:, 2 * C + h0 * HD: 2 * C + (h0 + NH) * HD]).astype(bf)
        wp_c = np.ascontiguousarray(Wp[hsl, :]).astype(bf)
        in_maps.append({
            "xtr": xtr_b[b], "wq": wq_c, "wk": wk_c, "wv": wv_c,
            "wp": wp_c, "tri": tri,
        })
    return in_maps, causal


def kernel(x, W_attn, W_proj, attn_mask, _trace=False):
    from concourse import bass_utils

    in_maps, causal = _prep_inputs(x, W_attn, W_proj, attn_mask)
    key = ("causal" if causal else "dense")
    if key not in _COMPILED:
        _COMPILED[key] = _build(causal)
    nc = _COMPILED[key]

    res = bass_utils.run_bass_kernel_spmd(
        nc, in_maps, core_ids=list(range(N_CORES)), trace=_trace)

    y = np.zeros((B, T, C), dtype=np.float32)
    for core in range(N_CORES):
        y[core // 4] += res.results[core]["y"].astype(np.float32)
    if _trace:
        kernel._last_results = res
    return y
